# revision 10
# baseline (speedup 1.0000x reference)
"""Trainium2 Bass kernel: scatter rho[b, i, j] -> out[b, fock_idx[i], fock_idx[j]].

Sharding: batch dim B across the 8 NeuronCores (pure data parallel). fock_idx is
known on the host at call time, so the scatter addressing is baked into the
compiled program as static DMA/compute access patterns.

Per-core algorithm (out is [D, D], zero except out[idx[i], idx[j]] = rho[i, j]):
  - The runtime hands the NEFF a zero-initialized ExternalOutput buffer, so
    only rows/columns that receive data are written.
  - fock_idx decomposes into 32 runs of 32 consecutive indices spanning
    [c0, c1).  Each rho row is expanded into a [span]-wide row in SBUF with
    the runs at their target offsets and zeros in the gaps; each out row-run
    is stored with one DMA touching columns [c0, c1) only.
  - The DMA engines (16 per core, ~22.5 GB/s each) are the bottleneck:
    ~4.2 MB of loads + ~8.3 MB of span stores = ~34.6 us of engine time.
    To keep them saturated end-to-end: loads pack 2 rho rows per SBUF
    partition (8 KB descriptors, double the per-queue packet throughput),
    are split over the qSP HWDGE ring and the qPool SWDGE ring, and are all
    issued up front; stores alternate between the qSP and qAct HWDGE rings.
  - W expansion buffers are zeroed once (Vector/GpSimd halves) and reused;
    gap columns stay zero across reuse because copies only write data
    columns.  Expansion copies are pair-merged (2 runs per instruction) and
    split across Vector, GpSimd and Scalar(Act activation-copy).
"""

import numpy as np

import concourse.bacc as bacc
import concourse.bass as bass
import concourse.mybir as mybir
from concourse import tile
from concourse.bass_utils import run_bass_kernel_spmd

N_CORES = 8
P = 128  # SBUF partitions


def _runs(dst, src):
    """Maximal runs where dst and src both advance by 1. Yields (d0, s0, len)."""
    out = []
    d0, s0, L = int(dst[0]), int(src[0]), 1
    for k in range(1, len(dst)):
        if int(dst[k]) == d0 + L and int(src[k]) == s0 + L:
            L += 1
        else:
            out.append((d0, s0, L))
            d0, s0, L = int(dst[k]), int(src[k]), 1
    out.append((d0, s0, L))
    return out


def _pair_runs(col_runs):
    """Group adjacent equal-length runs into stride-2 pairs.

    Returns a list of (dst0, src0, pair_dst_stride, pair_src_stride, n, L)
    where n is 1 or 2 repeats of an L-wide copy.
    """
    out = []
    k = 0
    while k < len(col_runs):
        d0, s0, L = col_runs[k]
        if k + 1 < len(col_runs) and col_runs[k + 1][2] == L:
            d1, s1, _ = col_runs[k + 1]
            out.append((d0, s0, d1 - d0, s1 - s0, 2, L))
            k += 2
        else:
            out.append((d0, s0, L, L, 1, L))
            k += 1
    return out


def _plan_tiles(n, row_runs):
    """Tile plan: (r0, rows, rpp, wbuf, pbase). rpp = rho rows per partition."""
    ok2 = n % 2 == 0 and all(s % 2 == 0 and L % 2 == 0 for _, s, L in row_runs)
    if n == 1024 and ok2:
        # Two W buffers cycled with no partition-sharing: sharing one W
        # between two in-flight tiles by partition ranges makes the tile
        # dependency tracker (which does not separate partition ranges)
        # chain every copy pair with completion semaphores.
        return [
            (0, 128, 2, "A", 0),
            (128, 128, 2, "B", 0),
            (256, 256, 2, "A", 0),
            (512, 256, 2, "B", 0),
            (768, 256, 2, "A", 0),
        ]
    tiles = []
    r0 = 0
    k = 0
    while r0 < n:
        rows = min(P, n - r0)
        tiles.append((r0, rows, 1, "A" if k % 2 == 0 else "B", 0))
        r0 += rows
        k += 1
    return tiles


def _build(idx, D, n):
    """Build the per-core Bass program with idx baked in."""
    f32 = mybir.dt.float32

    order = np.argsort(idx, kind="stable")
    col_runs = _runs(idx[order], order)  # (dst_col, src_col, len)
    c0 = min(r[0] for r in col_runs)
    c1 = max(r[0] + r[2] for r in col_runs)
    span = c1 - c0
    pairs = _pair_runs(col_runs)

    row_runs = _runs(idx, range(n))
    tiles = _plan_tiles(n, row_runs)
    rpp_max = max(t[2] for t in tiles)

    # Copy split: Vector and GpSimd only (Activation-engine copies measure
    # ~1.2 us each — 5x Vector — so Act only issues store DMAs).  Early
    # tiles lean harder on Vector because GpSimd's Q7 is busy issuing
    # SWDGE loads.
    def copy_engines(nc, t):
        npair = len(pairs)
        cut = (npair * 6) // 8 if t < 2 else (npair * 5) // 8
        return [(nc.vector, range(0, cut)), (nc.gpsimd, range(cut, npair))]

    # Loads: tiles 0/2 ride the SP HWDGE ring (its stores only queue behind
    # these two); the rest ride the Pool SWDGE ring.  All are issued up
    # front so the DMA engines stay fed before stores exist.
    def load_ring(nc, t):
        return nc.sync if t in (0, 2) else nc.gpsimd

    nc = bacc.Bacc("TRN2", target_bir_lowering=False, debug=False,
                   num_devices=N_CORES)
    rho = nc.dram_tensor("rho", [n, n], f32, kind="ExternalInput")
    out = nc.dram_tensor("out", [D, D], f32, kind="ExternalOutput")
    rho_flat = rho[:, :]

    with tile.TileContext(nc) as tc:
        with (
            tc.tile_pool(name="rp", bufs=1) as rp,
            tc.tile_pool(name="wp", bufs=1) as wp,
        ):
            ws = {
                "A": wp.tile([P, rpp_max * span], f32, name="WA"),
                "B": wp.tile([P, rpp_max * span], f32, name="WB"),
            }
            rs = [rp.tile([P, t[2] * n], f32, name=f"R{k}")
                  for k, t in enumerate(tiles)]
            first_b = next((t for t, td in enumerate(tiles)
                            if td[3] == "B"), None)

            def load_ap(t):
                r0, rows, rpp, _, _ = tiles[t]
                parts = rows // rpp
                src = bass.AP(rho_flat.tensor, rho_flat.offset + r0 * n,
                              [[rpp * n, parts], [1, rpp * n]])
                return rs[t][:parts, :], src

            # Issue loads up front: L0 on the SP HWDGE ring (its stores only
            # queue behind this one small load), the rest on the Pool SWDGE
            # ring.  W zero-fills interleave so WA is ready for tile 0 and
            # WB for tile 2 without delaying load issue.
            # WA is zeroed immediately (Vector/GpSimd halves); WB lazily on
            # Vector behind tile 0's copies — the front-loaded loads keep
            # the DMA engines fed until the first stores.
            d, s = load_ap(0)
            nc.sync.dma_start(d, s)
            nc.vector.memset(ws["A"][:, :span], 0.0)
            if rpp_max > 1:
                nc.gpsimd.memset(ws["A"][:, span:], 0.0)
            for t in range(1, len(tiles)):
                d, s = load_ap(t)
                load_ring(nc, t).dma_start(d, s)

            n_store = 0
            for t, (r0, rows, rpp, wname, pbase) in enumerate(tiles):
                parts = rows // rpp
                W = ws[wname]
                R = rs[t]
                if t == first_b:
                    nc.vector.memset(ws["B"][:, :span], 0.0)
                    if rpp_max > 1:
                        nc.vector.memset(ws["B"][:, span:], 0.0)

                # Expansion copies: runs placed at target offsets, 2 runs
                # per instruction where possible, all rpp sub-rows at once.
                for eng, rng in copy_engines(nc, t):
                    for k in rng:
                        d0, s0, ds, ss, cnt, L = pairs[k]
                        doff = W.offset + pbase * W.ap[0][0] + (d0 - c0)
                        soff = R.offset + s0
                        if cnt == 1:
                            dst = bass.AP(W.tensor, doff,
                                          [[W.ap[0][0], parts],
                                           [span, rpp], [1, L]])
                            src = bass.AP(R.tensor, soff,
                                          [[R.ap[0][0], parts],
                                           [n, rpp], [1, L]])
                        else:
                            dst = bass.AP(W.tensor, doff,
                                          [[W.ap[0][0], parts], [span, rpp],
                                           [ds, cnt], [1, L]])
                            src = bass.AP(R.tensor, soff,
                                          [[R.ap[0][0], parts], [n, rpp],
                                           [ss, cnt], [1, L]])
                        if eng is nc.scalar:
                            eng.copy(dst, src)
                        else:
                            eng.tensor_copy(dst, src)

                # Row-run stores for this tile, alternating HWDGE rings.
                for dr, sr, Lr in _runs(idx[r0:r0 + rows], range(rows)):
                    ring = nc.sync if n_store % 2 == 0 else nc.scalar
                    n_store += 1
                    if rpp == 1:
                        ring.dma_start(out[dr:dr + Lr, c0:c1],
                                       W[pbase + sr:pbase + sr + Lr, :span])
                        continue
                    if sr % rpp == 0 and Lr % rpp == 0:
                        p0 = pbase + sr // rpp
                        src = bass.AP(W.tensor, W.offset + p0 * W.ap[0][0],
                                      [[W.ap[0][0], Lr // rpp],
                                       [span, rpp], [1, span]])
                        ring.dma_start(out[dr:dr + Lr, c0:c1], src)
                    else:
                        for j in range(Lr):
                            p0 = pbase + (sr + j) // rpp
                            sub = (sr + j) % rpp
                            src = bass.AP(W.tensor,
                                          W.offset + p0 * W.ap[0][0]
                                          + sub * span,
                                          [[W.ap[0][0], 1], [1, span]])
                            r2 = nc.sync if n_store % 2 == 0 else nc.scalar
                            n_store += 1
                            r2.dma_start(out[dr + j:dr + j + 1, c0:c1], src)
    nc.compile()
    return nc


def kernel(input_state, fock_idx, fock_dim):
    input_state = np.asarray(input_state)
    idx = np.asarray(fock_idx).astype(np.int64)
    D = int(fock_dim)
    B, n, _ = input_state.shape

    nc = _build(idx, D, n)

    out = np.empty((B, D, D), dtype=input_state.dtype)
    for start in range(0, B, N_CORES):
        stop = min(start + N_CORES, B)
        in_maps = [
            {"rho": np.ascontiguousarray(input_state[b], dtype=np.float32)}
            for b in range(start, stop)
        ]
        res = run_bass_kernel_spmd(nc, in_maps,
                                   core_ids=list(range(stop - start)))
        for k, b in enumerate(range(start, stop)):
            out[b] = res.results[k]["out"]
    return out


# revision 11
# speedup vs baseline: 1.4106x; 1.4106x over previous
"""Trainium2 Bass kernel: scatter rho[b, i, j] -> out[b, fock_idx[i], fock_idx[j]].

Sharding: batch dim B across the 8 NeuronCores (pure data parallel). fock_idx is
known on the host at call time, so the scatter addressing is baked into the
compiled program as static DMA/compute access patterns.

Per-core algorithm (out is [D, D], zero except out[idx[i], idx[j]] = rho[i, j]):
  - The runtime hands the NEFF a zero-initialized ExternalOutput buffer, so
    only rows/columns that receive data are written.
  - fock_idx decomposes into 32 runs of 32 consecutive indices spanning
    [c0, c1).  Each rho row is expanded into a [span]-wide row in SBUF with
    the runs at their target offsets and zeros in the gaps; each out row-run
    is stored with one DMA touching columns [c0, c1) only.
  - The 16 DMA engines (~22.5 GB/s each, shared by all queues) are the
    bottleneck: ~4.2 MB of loads + ~8.3 MB of span stores = ~34.6 us of
    engine time.  To keep them saturated end-to-end, ALL loads are issued
    up front: tiles 0-1 on the Pool SWDGE queue (only two, because SWDGE
    descriptor generation runs on the Q7 cores and would serialize with
    GpSimd's copies), the rest interleaved onto the two HWDGE rings ahead
    of the stores.  A single SWDGE queue feeds 4 KB descriptors at only
    ~160 GB/s, which is why the baseline's loads starved the pipeline.
  - The W expansion buffers are memset once up front and reused cyclically:
    the gap columns stay zero across reuse because the per-tile copies only
    ever write the (fixed) data columns.  Expansion copies run on Vector
    (single runs) and GpSimd (pair-merged runs); stores alternate between
    the two HWDGE rings.
"""

import numpy as np

import concourse.bacc as bacc
import concourse.bass as bass
import concourse.mybir as mybir
from concourse import tile
from concourse.bass_utils import run_bass_kernel_spmd

N_CORES = 8
P = 128  # SBUF partitions
W_BUFS = 4


def _runs(dst, src):
    """Maximal runs where dst and src both advance by 1. Yields (d0, s0, len)."""
    out = []
    d0, s0, L = int(dst[0]), int(src[0]), 1
    for k in range(1, len(dst)):
        if int(dst[k]) == d0 + L and int(src[k]) == s0 + L:
            L += 1
        else:
            out.append((d0, s0, L))
            d0, s0, L = int(dst[k]), int(src[k]), 1
    out.append((d0, s0, L))
    return out


def _pair_runs(col_runs):
    """Group adjacent equal-length runs into stride-2 pairs.

    Returns a list of (dst0, src0, pair_dst_stride, pair_src_stride, n, L)
    where n is 1 or 2 repeats of an L-wide copy.
    """
    out = []
    k = 0
    while k < len(col_runs):
        d0, s0, L = col_runs[k]
        if k + 1 < len(col_runs) and col_runs[k + 1][2] == L:
            d1, s1, _ = col_runs[k + 1]
            out.append((d0, s0, d1 - d0, s1 - s0, 2, L))
            k += 2
        else:
            out.append((d0, s0, L, L, 1, L))
            k += 1
    return out


def _build(idx, D, n):
    """Build the per-core Bass program with idx baked in."""
    f32 = mybir.dt.float32

    # Column placement: process columns in sorted-index order so the SBUF row
    # image is written left to right; a run needs source columns contiguous too.
    order = np.argsort(idx, kind="stable")
    col_runs = _runs(idx[order], order)  # (dst_col, src_col, len)
    c0 = min(r[0] for r in col_runs)
    c1 = max(r[0] + r[2] for r in col_runs)
    span = c1 - c0

    # ~18/32 runs to Vector as singles; 14 to GpSimd as pair-merged copies.
    runs_v = [r for k, r in enumerate(col_runs) if k % 16 < 9]
    pairs_g = _pair_runs([r for k, r in enumerate(col_runs) if k % 16 >= 9])

    nc = bacc.Bacc("TRN2", target_bir_lowering=False, debug=False,
                   num_devices=N_CORES)
    rho = nc.dram_tensor("rho", [n, n], f32, kind="ExternalInput")
    out = nc.dram_tensor("out", [D, D], f32, kind="ExternalOutput")

    n_tiles = (n + P - 1) // P
    with tile.TileContext(nc) as tc:
        with (
            tc.tile_pool(name="rp", bufs=1) as rp,
            tc.tile_pool(name="wp", bufs=1) as wp,
        ):
            ws = [wp.tile([P, span], f32, name=f"W{k}") for k in range(W_BUFS)]
            memset_eng = [nc.vector if k % 2 == 0 else nc.gpsimd
                          for k in range(W_BUFS)]

            # One R buffer per tile — every load is in flight at once.
            Rts = [rp.tile([P, n], f32, name=f"R{t}") for t in range(n_tiles)]

            def issue_load(t, eng):
                r0 = t * P
                rows = min(P, n - r0)
                eng.dma_start(Rts[t][:rows, :], rho[r0:r0 + rows, :])

            # All loads up front: tiles 0-1 on the Pool SWDGE queue, the
            # rest split across the two HWDGE rings ahead of the stores.
            issue_load(0, nc.gpsimd)
            if n_tiles > 1:
                issue_load(1, nc.gpsimd)
            for t in range(2, n_tiles):
                issue_load(t, nc.sync if t % 2 == 0 else nc.scalar)

            # Memsets after load issue so they do not delay the queues.
            memset_eng[0].memset(ws[0][:], 0.0)
            memset_eng[1].memset(ws[1][:], 0.0)
            next_memset = 2

            n_store = 0
            for t in range(n_tiles):
                r0 = t * P
                rows = min(P, n - r0)
                R = Rts[t]

                W = ws[t % W_BUFS]
                for d0, s0, L in runs_v:
                    nc.vector.tensor_copy(
                        W[:rows, d0 - c0:d0 - c0 + L],
                        R[:rows, s0:s0 + L])
                for d0, s0, ds, ss, cnt, L in pairs_g:
                    dst = bass.AP(W.tensor, W.offset + (d0 - c0),
                                  [[W.ap[0][0], rows], [ds, cnt], [1, L]])
                    src = bass.AP(R.tensor, R.offset + s0,
                                  [[R.ap[0][0], rows], [ss, cnt], [1, L]])
                    nc.gpsimd.tensor_copy(dst, src)

                # Row runs within this tile: consecutive rho rows with
                # consecutive target rows share one store DMA, alternating
                # between the SP and ACT HWDGE rings.
                for dr, sr, L in _runs(idx[r0:r0 + rows], range(rows)):
                    ring = nc.sync if n_store % 2 == 0 else nc.scalar
                    n_store += 1
                    ring.dma_start(out[dr:dr + L, c0:c1], W[sr:sr + L, :])

                # Stagger the remaining one-time memsets behind early tiles.
                while next_memset < W_BUFS and next_memset <= t + 2:
                    memset_eng[next_memset].memset(ws[next_memset][:], 0.0)
                    next_memset += 1
    nc.compile()
    return nc


def kernel(input_state, fock_idx, fock_dim):
    input_state = np.asarray(input_state)
    idx = np.asarray(fock_idx).astype(np.int64)
    D = int(fock_dim)
    B, n, _ = input_state.shape

    nc = _build(idx, D, n)

    out = np.empty((B, D, D), dtype=input_state.dtype)
    for start in range(0, B, N_CORES):
        stop = min(start + N_CORES, B)
        in_maps = [
            {"rho": np.ascontiguousarray(input_state[b], dtype=np.float32)}
            for b in range(start, stop)
        ]
        res = run_bass_kernel_spmd(nc, in_maps,
                                   core_ids=list(range(stop - start)))
        for k, b in enumerate(range(start, stop)):
            out[b] = res.results[k]["out"]
    return out


# revision 14
# speedup vs baseline: 1.5766x; 1.1176x over previous
"""Trainium2 Bass kernel: scatter rho[b, i, j] -> out[b, fock_idx[i], fock_idx[j]].

Sharding: batch dim B across the 8 NeuronCores (pure data parallel). fock_idx is
known on the host at call time, so the scatter addressing is baked into the
compiled program as static DMA/compute access patterns.

Per-core algorithm (out is [D, D], zero except out[idx[i], idx[j]] = rho[i, j]):
  - The runtime hands the NEFF a zero-initialized ExternalOutput buffer, so
    only rows/columns that receive data are written.
  - fock_idx decomposes into 32 runs of 32 consecutive indices spanning
    [c0, c1).  Each rho row is expanded into a [span]-wide row in SBUF with
    the runs at their target offsets and zeros in the gaps; each out row-run
    is stored with one DMA touching columns [c0, c1) only.
  - The 16 DMA engines (~22.5 GB/s each, shared by all queues) are the
    bottleneck: ~4.2 MB of loads + ~8.3 MB of span stores = ~34.6 us of
    engine time.  To keep them saturated end-to-end, ALL loads are issued
    up front: tiles 0-1 on the Pool SWDGE queue (only two, because SWDGE
    descriptor generation runs on the Q7 cores and would serialize with
    GpSimd's copies), the rest interleaved onto the two HWDGE rings ahead
    of the stores.  A single SWDGE queue feeds 4 KB descriptors at only
    ~160 GB/s, which is why the baseline's loads starved the pipeline.
  - The W expansion buffers are memset once up front and reused cyclically:
    the gap columns stay zero across reuse because the per-tile copies only
    ever write the (fixed) data columns.  Expansion copies run on Vector
    (single runs) and GpSimd (pair-merged runs); stores alternate between
    the two HWDGE rings.
"""

import numpy as np

import concourse.bacc as bacc
import concourse.bass as bass
import concourse.mybir as mybir
from concourse import tile
from concourse.bass_utils import run_bass_kernel_spmd

N_CORES = 8
P = 128  # SBUF partitions
W_BUFS = 4


def _runs(dst, src):
    """Maximal runs where dst and src both advance by 1. Yields (d0, s0, len)."""
    out = []
    d0, s0, L = int(dst[0]), int(src[0]), 1
    for k in range(1, len(dst)):
        if int(dst[k]) == d0 + L and int(src[k]) == s0 + L:
            L += 1
        else:
            out.append((d0, s0, L))
            d0, s0, L = int(dst[k]), int(src[k]), 1
    out.append((d0, s0, L))
    return out


def _pair_runs(col_runs):
    """Group adjacent equal-length runs into stride-2 pairs.

    Returns a list of (dst0, src0, pair_dst_stride, pair_src_stride, n, L)
    where n is 1 or 2 repeats of an L-wide copy.
    """
    out = []
    k = 0
    while k < len(col_runs):
        d0, s0, L = col_runs[k]
        if k + 1 < len(col_runs) and col_runs[k + 1][2] == L:
            d1, s1, _ = col_runs[k + 1]
            out.append((d0, s0, d1 - d0, s1 - s0, 2, L))
            k += 2
        else:
            out.append((d0, s0, L, L, 1, L))
            k += 1
    return out


def _build(idx, D, n):
    """Build the per-core Bass program with idx baked in."""
    f32 = mybir.dt.float32

    # Column placement: process columns in sorted-index order so the SBUF row
    # image is written left to right; a run needs source columns contiguous too.
    order = np.argsort(idx, kind="stable")
    col_runs = _runs(idx[order], order)  # (dst_col, src_col, len)
    c0 = min(r[0] for r in col_runs)
    c1 = max(r[0] + r[2] for r in col_runs)
    span = c1 - c0

    # All copies pair-merged (2 runs per instruction); 10/16 pairs to
    # Vector, 6/16 to GpSimd so the per-tile staging pace (~2.3 us) feeds
    # stores faster than the DMA engines drain them.
    all_pairs = _pair_runs(col_runs)
    ncut = (len(all_pairs) * 10 + 15) // 16
    pairs_v = all_pairs[:ncut]
    pairs_g = all_pairs[ncut:]

    nc = bacc.Bacc("TRN2", target_bir_lowering=False, debug=False,
                   num_devices=N_CORES)
    rho = nc.dram_tensor("rho", [n, n], f32, kind="ExternalInput")
    out = nc.dram_tensor("out", [D, D], f32, kind="ExternalOutput")

    n_tiles = (n + P - 1) // P
    with tile.TileContext(nc) as tc:
        with (
            tc.tile_pool(name="rp", bufs=1) as rp,
            tc.tile_pool(name="wp", bufs=1) as wp,
        ):
            ws = [wp.tile([P, span], f32, name=f"W{k}") for k in range(W_BUFS)]
            memset_eng = [nc.vector if k % 2 == 0 else nc.gpsimd
                          for k in range(W_BUFS)]

            # One R buffer per tile — every load is in flight at once.
            Rts = [rp.tile([P, n], f32, name=f"R{t}") for t in range(n_tiles)]

            def issue_load(t, eng):
                r0 = t * P
                rows = min(P, n - r0)
                eng.dma_start(Rts[t][:rows, :], rho[r0:r0 + rows, :])

            # All loads up front, first in each HWDGE ring's FIFO (before
            # any stores).  No SWDGE: its descriptor generation runs on the
            # Q7 cores and both serializes with GpSimd's copies and feeds
            # packets at only ~115 GB/s.
            for t in range(n_tiles):
                issue_load(t, nc.sync if t % 2 == 0 else nc.scalar)

            # Memsets after load issue so they do not delay the queues.
            memset_eng[0].memset(ws[0][:], 0.0)
            memset_eng[1].memset(ws[1][:], 0.0)
            next_memset = 2

            n_store = 0
            for t in range(n_tiles):
                r0 = t * P
                rows = min(P, n - r0)
                R = Rts[t]

                W = ws[t % W_BUFS]
                for eng, plist in ((nc.vector, pairs_v), (nc.gpsimd, pairs_g)):
                    for d0, s0, ds, ss, cnt, L in plist:
                        dst = bass.AP(W.tensor, W.offset + (d0 - c0),
                                      [[W.ap[0][0], rows], [ds, cnt], [1, L]])
                        src = bass.AP(R.tensor, R.offset + s0,
                                      [[R.ap[0][0], rows], [ss, cnt], [1, L]])
                        eng.tensor_copy(dst, src)

                # Row runs within this tile: consecutive rho rows with
                # consecutive target rows share one store DMA, alternating
                # between the SP and ACT HWDGE rings.
                for dr, sr, L in _runs(idx[r0:r0 + rows], range(rows)):
                    ring = nc.sync if n_store % 2 == 0 else nc.scalar
                    n_store += 1
                    ring.dma_start(out[dr:dr + L, c0:c1], W[sr:sr + L, :])

                # Stagger the remaining one-time memsets behind early tiles.
                while next_memset < W_BUFS and next_memset <= t + 2:
                    memset_eng[next_memset].memset(ws[next_memset][:], 0.0)
                    next_memset += 1
    nc.compile()
    return nc


def kernel(input_state, fock_idx, fock_dim):
    input_state = np.asarray(input_state)
    idx = np.asarray(fock_idx).astype(np.int64)
    D = int(fock_dim)
    B, n, _ = input_state.shape

    nc = _build(idx, D, n)

    out = np.empty((B, D, D), dtype=input_state.dtype)
    for start in range(0, B, N_CORES):
        stop = min(start + N_CORES, B)
        in_maps = [
            {"rho": np.ascontiguousarray(input_state[b], dtype=np.float32)}
            for b in range(start, stop)
        ]
        res = run_bass_kernel_spmd(nc, in_maps,
                                   core_ids=list(range(stop - start)))
        for k, b in enumerate(range(start, stop)):
            out[b] = res.results[k]["out"]
    return out
